# revision 20
# baseline (speedup 1.0000x reference)
"""Trainium2 Bass kernel for nn_Decoder_4784593567814 (sparse block-local attention decoder layer).

Sharding: data-parallel over the 16384 tokens -> 8 cores x 2048 tokens
(8 independent 256-token causal windows per core). Weights replicated.
No collectives.
"""
import sys

sys.path.insert(0, "/opt/trn_rl_repo")

import numpy as np
import ml_dtypes

import concourse.bass as bass
import concourse.mybir as mybir
import concourse.tile as tile
from concourse.bacc import Bacc
from concourse.bass_utils import run_bass_kernel_spmd

F32 = mybir.dt.float32
BF16 = mybir.dt.bfloat16
AF = mybir.ActivationFunctionType
ALU = mybir.AluOpType

B, T, E, H, W = 4, 4096, 1024, 16, 256
DH = E // H            # 64
NCORES = 8
S = (B * T) // NCORES  # 2048 tokens per core
NWIN = S // W          # 8 windows per core
P = 128
EC = E // P            # 8 chunks of the E contraction
FC = (4 * E) // P      # 32 chunks of the hidden dim
JTQK = (2 * E) // P    # 16 feature tiles of the packed q,k projection
EPS = 1e-5
SCALE = 1.0 / 8.0      # 1/sqrt(DH)
NEG = -1e9

_NC_CACHE = {}


def _build_nc():
    nc = Bacc("TRN2")

    x_in = nc.dram_tensor("x", [S, E], F32, kind="ExternalInput")
    wqk = nc.dram_tensor("wqk", [E, 2 * E], BF16, kind="ExternalInput")
    wv = nc.dram_tensor("wv", [E, E], BF16, kind="ExternalInput")
    wo = nc.dram_tensor("wo", [E, E], BF16, kind="ExternalInput")
    w1 = nc.dram_tensor("w1", [E, 4 * E], BF16, kind="ExternalInput")
    w2 = nc.dram_tensor("w2", [4 * E, E], BF16, kind="ExternalInput")
    bqk = nc.dram_tensor("bqk", [2 * E], F32, kind="ExternalInput")
    b1e = nc.dram_tensor("b1e", [4 * E], F32, kind="ExternalInput")
    bvr = nc.dram_tensor("bvr", [1, E], BF16, kind="ExternalInput")
    bor = nc.dram_tensor("bor", [1, E], BF16, kind="ExternalInput")
    b2r = nc.dram_tensor("b2r", [1, E], BF16, kind="ExternalInput")
    maskadd = nc.dram_tensor("maskadd", [W, W], BF16, kind="ExternalInput")
    id128 = nc.dram_tensor("id128", [P, P], BF16, kind="ExternalInput")
    y_out = nc.dram_tensor("y", [S, E], F32, kind="ExternalOutput")
    x2s = nc.dram_tensor("x2s", [S, E], F32)

    with tile.TileContext(nc) as tc:
        with tc.tile_pool(name="consts", bufs=1) as consts:
            sb_bqk = consts.tile([P, JTQK], F32)
            nc.sync.dma_start(sb_bqk, bqk.rearrange("(jt p) -> p jt", p=P))
            sb_b1 = consts.tile([P, FC], F32)
            nc.sync.dma_start(sb_b1, b1e.rearrange("(f p) -> p f", p=P))
            sb_bvr = consts.tile([1, E], BF16)
            nc.sync.dma_start(sb_bvr, bvr[:, :])
            sb_bor = consts.tile([1, E], BF16)
            nc.sync.dma_start(sb_bor, bor[:, :])
            sb_b2r = consts.tile([1, E], BF16)
            nc.sync.dma_start(sb_b2r, b2r[:, :])
            sb_mask = consts.tile([P, 2, W], BF16)
            nc.sync.dma_start(sb_mask, maskadd.rearrange("(qt p) k -> p qt k", p=P))
            sb_id = consts.tile([P, P], BF16)
            nc.sync.dma_start(sb_id, id128[:, :])
            eps_t = consts.tile([P, 1], F32)
            nc.vector.memset(eps_t, EPS)
            ones_t = consts.tile([1, P], BF16)
            nc.vector.memset(ones_t, 1.0)

            # ---------------- stage B: attention block ----------------
            with (
                tc.tile_pool(name="wB", bufs=1) as wB,
                tc.tile_pool(name="xpool", bufs=6) as xpool,
                tc.tile_pool(name="sbB2", bufs=2) as sbB2,
                tc.tile_pool(name="sbB3", bufs=3) as sbB3,
                tc.tile_pool(name="sbB4", bufs=4) as sbB4,
                tc.tile_pool(name="sbB18", bufs=18) as sbB18,
                tc.tile_pool(name="ps256", bufs=3, space="PSUM") as ps256,
                tc.tile_pool(name="psmix", bufs=5, space="PSUM") as psmix,
            ):
                # qk weights in 4 column chunks (separate tiles) so the first
                # QK matmuls start after ~1MB instead of the full 8MB stream
                wqk_r = wqk.rearrange("(ec p) j -> p ec j", p=P)
                sb_wqk_c = []
                for c in range(4):
                    sb_wqk = wB.tile([P, EC, 512], BF16, name=f"sb_wqk{c}")
                    nc.sync.dma_start(sb_wqk, wqk_r[:, :, c * 512:(c + 1) * 512])
                    sb_wqk_c.append(sb_wqk)
                sb_wv = wB.tile([P, EC, E], BF16)
                nc.sync.dma_start(sb_wv, wv.rearrange("(ec p) j -> p ec j", p=P))
                sb_wo = wB.tile([P, EC, E], BF16)
                nc.sync.dma_start(sb_wo, wo.rearrange("(ec p) j -> p ec j", p=P))

                def ln1_transpose(win):
                    """Load x, LN1, transpose -> (x_tiles, x_T). Emitted mid
                    previous window so the chain overlaps attention."""
                    t0 = win * W
                    x_tiles = []
                    x_T = sbB2.tile([P, EC, W], BF16, tag="xT", name=f"x_T{win}")
                    for tt in range(2):
                        xt = xpool.tile([P, E], F32, tag="x", name=f"x{win}_{tt}")
                        nc.scalar.dma_start(xt, x_in[t0 + tt * P:t0 + (tt + 1) * P, :])
                        x_tiles.append(xt)
                        stats = sbB3.tile([P, 2, 6], F32, tag="bst")
                        for sg in range(2):
                            nc.vector.bn_stats(stats[:, sg], xt[:, sg * 512:(sg + 1) * 512])
                        mv = sbB3.tile([P, 2], F32, tag="mv")
                        nc.vector.bn_aggr(mv, stats)
                        std = sbB3.tile([P, 1], F32, tag="std")
                        nc.scalar.activation(std, mv[:, 1:2], AF.Sqrt, bias=eps_t)
                        rstd = sbB3.tile([P, 1], F32, tag="rstd")
                        nc.vector.reciprocal(rstd, std)
                        xnb = sbB4.tile([P, E], BF16, tag="xnb")
                        nc.vector.tensor_scalar(
                            out=xnb, in0=xt, scalar1=mv[:, 0:1], scalar2=rstd,
                            op0=ALU.subtract, op1=ALU.mult,
                        )
                        for ec2 in range(0, EC, 2):
                            ptr = psmix.tile([P, W], BF16, tag="mix", name="ptr_x")
                            nc.tensor.transpose(ptr[:, :P], xnb[:, ec2 * P:(ec2 + 1) * P], sb_id)
                            nc.tensor.transpose(ptr[:, P:], xnb[:, (ec2 + 1) * P:(ec2 + 2) * P], sb_id)
                            nc.vector.tensor_copy(
                                out=x_T[:, ec2:ec2 + 2, tt * P:(tt + 1) * P],
                                in_=ptr.rearrange("p (e q) -> p e q", e=2),
                            )
                    return x_tiles, x_T

                def qk_proj(x_T, win):
                    qkT = sbB2.tile([P, JTQK, W], BF16, tag="qkT", name=f"qkT{win}")
                    # q tile then matching k tile so scores of head pair h2 can
                    # start after 2 copies instead of 9
                    for idx in range(H // 2):
                        for jt in (idx, (H // 2) + idx):
                            ps = ps256.tile([P, W], F32, tag="mm256", name="ps_qk")
                            wq_c = sb_wqk_c[jt // 4]
                            jo = (jt % 4) * P
                            for ec in range(EC):
                                nc.tensor.matmul(
                                    ps, wq_c[:, ec, jo:jo + P], x_T[:, ec, :],
                                    start=(ec == 0), stop=(ec == EC - 1),
                                )
                            nc.scalar.activation(
                                qkT[:, jt, :], ps, AF.Prelu, bias=sb_bqk[:, jt:jt + 1], alpha=1.0,
                            )
                    return qkT

                def v_proj(x_T, win):
                    v_sb = sbB2.tile([P, 2, E], BF16, tag="v", name=f"v{win}")
                    for tt in range(2):
                        for jb in range(2):
                            ps = psmix.tile([P, 512], F32, tag="mix", name="ps_big")
                            for ec in range(EC):
                                nc.tensor.matmul(
                                    ps, x_T[:, ec, tt * P:(tt + 1) * P],
                                    sb_wv[:, ec, jb * 512:(jb + 1) * 512],
                                    start=(ec == 0), stop=False,
                                )
                            nc.tensor.matmul(
                                ps, ones_t, sb_bvr[0:1, jb * 512:(jb + 1) * 512],
                                start=False, stop=True,
                            )
                            nc.scalar.activation(v_sb[:, tt, jb * 512:(jb + 1) * 512], ps, AF.Copy)
                    return v_sb

                def attn_phase1(qkT):
                    # causal structure: qt0 rows attend only k<128 (upper k
                    # half fully masked -> zeros); qt1 rows attend all 256 with
                    # a triangular mask on the k 128..255 diagonal block only.
                    e_ns = []
                    for h in range(H):
                        h2, sub = h // 2, h % 2
                        poff = sub * DH
                        e_n = sbB18.tile([P, 2, W], BF16, tag="en", name=f"e_n{h}")
                        lsum = sbB18.tile([P, 2], F32, tag="l", name=f"l{h}")
                        ps0 = ps256.tile([P, W], F32, tag="mm256", name="ps_s0")
                        nc.tensor.matmul(ps0[:, :P], sb_id, sb_mask[:, 0, :P], start=True, stop=False)
                        nc.tensor.matmul(
                            ps0[:, :P],
                            qkT[poff:poff + DH, h2, :P],
                            qkT[poff:poff + DH, (H // 2) + h2, :P],
                            start=False, stop=True,
                        )
                        nc.scalar.activation(
                            e_n[:, 0, :P], ps0[:, :P], AF.Exp, scale=SCALE,
                            accum_out=lsum[:, 0:1],
                        )
                        nc.gpsimd.memset(e_n[:, 0, P:], 0.0)
                        ps1 = ps256.tile([P, W], F32, tag="mm256", name="ps_s1")
                        nc.tensor.matmul(
                            ps1,
                            qkT[poff:poff + DH, h2, P:],
                            qkT[poff:poff + DH, (H // 2) + h2, :],
                            start=True, stop=False,
                        )
                        nc.tensor.matmul(ps1[:, P:], sb_id, sb_mask[:, 0, :P], start=False, stop=True)
                        nc.scalar.activation(
                            e_n[:, 1, :], ps1, AF.Exp, scale=SCALE,
                            accum_out=lsum[:, 1:2],
                        )
                        rl = sbB18.tile([P, 2], F32, tag="rl", name=f"rl{h}")
                        nc.vector.reciprocal(rl, lsum)
                        nc.vector.tensor_scalar_mul(e_n[:, 0, :P], e_n[:, 0, :P], rl[:, 0:1])
                        nc.vector.tensor_scalar_mul(e_n[:, 1, :], e_n[:, 1, :], rl[:, 1:2])
                        e_ns.append(e_n)
                    return e_ns

                def attn_phase2(e_ns, v_sb):
                    eTs = []
                    for h in range(H):
                        e_n = e_ns[h]
                        eT = sbB18.tile([P, 2, W], BF16, tag="eT", name=f"eT{h}")
                        nc.gpsimd.memset(eT[:, 1, :P], 0.0)
                        ptr = psmix.tile([P, W], BF16, tag="mix", name="ptr_e")
                        nc.tensor.transpose(ptr[:, :P], e_n[:, 0, :P], sb_id)
                        nc.tensor.transpose(ptr[:, P:], e_n[:, 1, :P], sb_id)
                        nc.vector.tensor_copy(out=eT[:, 0, :], in_=ptr)
                        ptr2 = psmix.tile([P, W], BF16, tag="mix", name="ptr_e2")
                        nc.tensor.transpose(ptr2[:, :P], e_n[:, 1, P:], sb_id)
                        nc.vector.tensor_copy(out=eT[:, 1, P:], in_=ptr2[:, :P])
                        eTs.append(eT)
                    oT = sbB2.tile([P, EC, W], BF16, tag="oT")
                    for h2 in range(H // 2):
                        po = psmix.tile([P, W], F32, tag="mix", name="po")
                        for sub in range(2):
                            h = h2 * 2 + sub
                            poff = sub * DH
                            for kc in range(2):
                                nc.tensor.matmul(
                                    po[poff:poff + DH, :],
                                    v_sb[:, kc, h * DH:(h + 1) * DH],
                                    eTs[h][:, kc, :],
                                    start=(kc == 0), stop=(kc == 1),
                                    tile_position=(0, poff),
                                )
                        nc.vector.tensor_copy(out=oT[:, h2, :], in_=po)
                    return oT

                def out_proj(oT, x_tiles, win):
                    t0 = win * W
                    for tt in range(2):
                        x2t = xpool.tile([P, E], F32, tag="x2")
                        for jb in range(2):
                            ps = psmix.tile([P, 512], F32, tag="mix", name="ps_big")
                            for ec in range(EC):
                                nc.tensor.matmul(
                                    ps, oT[:, ec, tt * P:(tt + 1) * P],
                                    sb_wo[:, ec, jb * 512:(jb + 1) * 512],
                                    start=(ec == 0), stop=False,
                                )
                            nc.tensor.matmul(
                                ps, ones_t, sb_bor[0:1, jb * 512:(jb + 1) * 512],
                                start=False, stop=True,
                            )
                            nc.vector.tensor_tensor(
                                out=x2t[:, jb * 512:(jb + 1) * 512], in0=ps,
                                in1=x_tiles[tt][:, jb * 512:(jb + 1) * 512], op=ALU.add,
                            )
                        nc.scalar.dma_start(x2s[t0 + tt * P:t0 + (tt + 1) * P, :], x2t)

                # steady state: phase1(w) first (highest priority), then
                # QK/V(w+1) as dense PE filler that the scheduler slots into
                # the softmax-latency stalls, then phase2(w) by which time all
                # of window w's attn weights are ready.
                ln_cur = ln1_transpose(0)
                qkT_cur = qk_proj(ln_cur[1], 0)
                v_cur = v_proj(ln_cur[1], 0)
                ln_nxt = ln1_transpose(1)
                for win in range(NWIN):
                    e_ns = attn_phase1(qkT_cur)
                    if win + 1 < NWIN:
                        qkT_nxt = qk_proj(ln_nxt[1], win + 1)
                        v_nxt = v_proj(ln_nxt[1], win + 1)
                        ln_n2 = ln1_transpose(win + 2) if win + 2 < NWIN else None
                    oT = attn_phase2(e_ns, v_cur)
                    out_proj(oT, ln_cur[0], win)
                    if win + 1 < NWIN:
                        ln_cur, qkT_cur, v_cur, ln_nxt = ln_nxt, qkT_nxt, v_nxt, ln_n2

            # ---------------- stage C: MLP block ----------------
            with (
                tc.tile_pool(name="wC", bufs=1) as wC,
                tc.tile_pool(name="x2pool", bufs=4) as x2pool,
                tc.tile_pool(name="sbC2", bufs=2) as sbC2,
                tc.tile_pool(name="sbC3", bufs=3) as sbC3,
                tc.tile_pool(name="sbC4", bufs=4) as sbC4,
                tc.tile_pool(name="psy", bufs=4, space="PSUM") as psy,
                tc.tile_pool(name="psu", bufs=2, space="PSUM") as psu_p,
                tc.tile_pool(name="pstr2", bufs=2, space="PSUM") as pstr2,
            ):
                sb_w1 = wC.tile([P, EC, 4 * E], BF16)
                nc.sync.dma_start(sb_w1, w1.rearrange("(ec p) f -> p ec f", p=P))
                sb_w2 = wC.tile([P, FC, E], BF16)
                nc.sync.dma_start(sb_w2, w2.rearrange("(f p) j -> p f j", p=P))

                def ln2_transpose(pair):
                    """Load x2, LN2, transpose -> (x2_tiles, h_T). Emitted one
                    pair ahead so the LN chain overlaps the previous f-loop."""
                    t0 = pair * W
                    x2_tiles = []
                    h_T = sbC2.tile([P, EC, W], BF16, tag="hT", name=f"h_T{pair}")
                    for tt in range(2):
                        x2t = x2pool.tile([P, E], F32, tag="x2c", name=f"x2c{pair}_{tt}")
                        nc.scalar.dma_start(x2t, x2s[t0 + tt * P:t0 + (tt + 1) * P, :])
                        x2_tiles.append(x2t)
                        stats = sbC3.tile([P, 2, 6], F32, tag="bst")
                        for sg in range(2):
                            nc.vector.bn_stats(stats[:, sg], x2t[:, sg * 512:(sg + 1) * 512])
                        mv = sbC3.tile([P, 2], F32, tag="mv")
                        nc.vector.bn_aggr(mv, stats)
                        std = sbC3.tile([P, 1], F32, tag="std")
                        nc.scalar.activation(std, mv[:, 1:2], AF.Sqrt, bias=eps_t)
                        rstd = sbC3.tile([P, 1], F32, tag="rstd")
                        nc.vector.reciprocal(rstd, std)
                        hnb = sbC4.tile([P, E], BF16, tag="hnb")
                        nc.vector.tensor_scalar(
                            out=hnb, in0=x2t, scalar1=mv[:, 0:1], scalar2=rstd,
                            op0=ALU.subtract, op1=ALU.mult,
                        )
                        for ec2 in range(0, EC, 2):
                            ptr = pstr2.tile([P, 2 * P], BF16, tag="tr2")
                            nc.tensor.transpose(ptr[:, :P], hnb[:, ec2 * P:(ec2 + 1) * P], sb_id)
                            nc.tensor.transpose(ptr[:, P:], hnb[:, (ec2 + 1) * P:(ec2 + 2) * P], sb_id)
                            nc.vector.tensor_copy(
                                out=h_T[:, ec2:ec2 + 2, tt * P:(tt + 1) * P],
                                in_=ptr.rearrange("p (e q) -> p e q", e=2),
                            )
                    return x2_tiles, h_T

                def mlp1(f, h_T):
                    psu = psu_p.tile([P, W], F32, tag="u", name=f"psu{f}")
                    for ec in range(EC):
                        nc.tensor.matmul(
                            psu, sb_w1[:, ec, f * P:(f + 1) * P], h_T[:, ec, :],
                            start=(ec == 0), stop=(ec == EC - 1),
                        )
                    g = sbC3.tile([P, W], BF16, tag="g", name=f"g{f}")
                    nc.scalar.activation(g, psu, AF.Gelu_apprx_tanh, bias=sb_b1[:, f:f + 1])
                    return g

                pending = ln2_transpose(0)
                for pair in range(NWIN):
                    t0 = pair * W
                    x2_tiles, h_T = pending

                    y_ps = [psy.tile([P, 512], F32, tag="y", name=f"y_ps{i}") for i in range(4)]
                    # software-pipelined f loop: MLP1(f+1) issued before MLP2(f);
                    # next pair's LN2+transpose chain emitted mid-loop so its
                    # DVE work hides under this pair's matmuls
                    g = mlp1(0, h_T)
                    for f in range(FC):
                        if f == 4 and pair + 1 < NWIN:
                            pending = ln2_transpose(pair + 1)
                        g_next = mlp1(f + 1, h_T) if f + 1 < FC else None
                        for tt in range(2):
                            for jb in range(2):
                                nc.tensor.matmul(
                                    y_ps[tt * 2 + jb], g[:, tt * P:(tt + 1) * P],
                                    sb_w2[:, f, jb * 512:(jb + 1) * 512],
                                    start=(f == 0), stop=False,
                                )
                        g = g_next
                    for tt in range(2):
                        yt = sbC3.tile([P, E], F32, tag="yt")
                        for jb in range(2):
                            nc.tensor.matmul(
                                y_ps[tt * 2 + jb], ones_t, sb_b2r[0:1, jb * 512:(jb + 1) * 512],
                                start=False, stop=True,
                            )
                            nc.vector.tensor_tensor(
                                out=yt[:, jb * 512:(jb + 1) * 512], in0=y_ps[tt * 2 + jb],
                                in1=x2_tiles[tt][:, jb * 512:(jb + 1) * 512], op=ALU.add,
                            )
                        nc.scalar.dma_start(y_out[t0 + tt * P:t0 + (tt + 1) * P, :], yt)

    nc.finalize()
    return nc


def get_nc():
    if "nc" not in _NC_CACHE:
        _NC_CACHE["nc"] = _build_nc()
    return _NC_CACHE["nc"]


def _prep_inputs(x, ln1_g, ln1_b, ln2_g, ln2_b, w_in, b_in, w_out, b_out, w1, b1, w2, b2):
    bf = ml_dtypes.bfloat16
    f32 = np.float32

    x = np.asarray(x, f32)
    ln1_g = np.asarray(ln1_g, f32)
    ln1_b = np.asarray(ln1_b, f32)
    ln2_g = np.asarray(ln2_g, f32)
    ln2_b = np.asarray(ln2_b, f32)
    w_in = np.asarray(w_in, f32)
    b_in = np.asarray(b_in, f32)
    w_out = np.asarray(w_out, f32)
    b_out = np.asarray(b_out, f32)
    w1 = np.asarray(w1, f32)
    b1 = np.asarray(b1, f32)
    w2 = np.asarray(w2, f32)
    b2 = np.asarray(b2, f32)

    # fold LN1 affine into the in-projection, LN2 affine into the MLP first layer
    wqk_r = w_in[:2 * E] * ln1_g[None, :]          # [2E, E]
    wv_r = w_in[2 * E:] * ln1_g[None, :]           # [E, E]
    bqk_eff = b_in[:2 * E] + w_in[:2 * E] @ ln1_b  # [2E]
    bv_eff = b_in[2 * E:] + w_in[2 * E:] @ ln1_b   # [E]
    w1_r = w1 * ln2_g[None, :]                     # [4E, E]
    b1_eff = b1 + w1 @ ln2_b                       # [4E]

    tril = np.tril(np.ones((W, W), bool))
    maskadd = np.where(tril, 0.0, NEG).astype(f32)

    shared = {
        "wqk": np.ascontiguousarray(wqk_r.T).astype(bf),
        "wv": np.ascontiguousarray(wv_r.T).astype(bf),
        "wo": np.ascontiguousarray(w_out.T).astype(bf),
        "w1": np.ascontiguousarray(w1_r.T).astype(bf),
        "w2": np.ascontiguousarray(w2.T).astype(bf),
        "bqk": np.ascontiguousarray(bqk_eff),
        "b1e": np.ascontiguousarray(b1_eff),
        "bvr": bv_eff.reshape(1, E).astype(bf),
        "bor": b_out.reshape(1, E).astype(bf),
        "b2r": b2.reshape(1, E).astype(bf),
        "maskadd": maskadd.astype(bf),
        "id128": np.eye(P, dtype=f32).astype(bf),
    }
    x_flat = np.ascontiguousarray(x.reshape(B * T, E))
    in_maps = []
    for i in range(NCORES):
        m = dict(shared)
        m["x"] = np.ascontiguousarray(x_flat[i * S:(i + 1) * S])
        in_maps.append(m)
    return in_maps


def kernel(**inputs) -> np.ndarray:
    in_maps = _prep_inputs(**inputs)
    nc = get_nc()
    res = run_bass_kernel_spmd(nc, in_maps, core_ids=list(range(NCORES)))
    out = np.concatenate([res.results[i]["y"] for i in range(NCORES)], axis=0)
    return np.ascontiguousarray(out.reshape(B, T, E))


# revision 24
# speedup vs baseline: 1.2822x; 1.2822x over previous
"""Trainium2 Bass kernel for nn_Decoder_4784593567814 (sparse block-local attention decoder layer).

Sharding: data-parallel over the 16384 tokens -> 8 cores x 2048 tokens
(8 independent 256-token causal windows per core). Weights replicated.
No collectives.
"""
import sys

sys.path.insert(0, "/opt/trn_rl_repo")

import numpy as np
import ml_dtypes

import concourse.bass as bass
import concourse.mybir as mybir
import concourse.tile as tile
from concourse.bacc import Bacc
from concourse.bass_utils import run_bass_kernel_spmd

F32 = mybir.dt.float32
BF16 = mybir.dt.bfloat16
AF = mybir.ActivationFunctionType
ALU = mybir.AluOpType

B, T, E, H, W = 4, 4096, 1024, 16, 256
DH = E // H            # 64
NCORES = 8
S = (B * T) // NCORES  # 2048 tokens per core
NWIN = S // W          # 8 windows per core
P = 128
EC = E // P            # 8 chunks of the E contraction
FC = (4 * E) // P      # 32 chunks of the hidden dim
JTQK = (2 * E) // P    # 16 feature tiles of the packed q,k projection
EPS = 1e-5
SCALE = 1.0 / 8.0      # 1/sqrt(DH)
NEG = -1e9

_NC_CACHE = {}


def _build_nc():
    nc = Bacc("TRN2")

    x_in = nc.dram_tensor("x", [S, E], F32, kind="ExternalInput")
    wqk = nc.dram_tensor("wqk", [E, 2 * E], BF16, kind="ExternalInput")
    wv = nc.dram_tensor("wv", [E, E], BF16, kind="ExternalInput")
    wo = nc.dram_tensor("wo", [E, E], BF16, kind="ExternalInput")
    w1 = nc.dram_tensor("w1", [E, 4 * E], BF16, kind="ExternalInput")
    w2 = nc.dram_tensor("w2", [4 * E, E], BF16, kind="ExternalInput")
    bqk = nc.dram_tensor("bqk", [2 * E], F32, kind="ExternalInput")
    b1e = nc.dram_tensor("b1e", [4 * E], F32, kind="ExternalInput")
    bvr = nc.dram_tensor("bvr", [1, E], BF16, kind="ExternalInput")
    bor = nc.dram_tensor("bor", [1, E], BF16, kind="ExternalInput")
    b2r = nc.dram_tensor("b2r", [1, E], BF16, kind="ExternalInput")
    maskadd = nc.dram_tensor("maskadd", [W, W], BF16, kind="ExternalInput")
    id128 = nc.dram_tensor("id128", [P, P], BF16, kind="ExternalInput")
    y_out = nc.dram_tensor("y", [S, E], F32, kind="ExternalOutput")
    x2s = nc.dram_tensor("x2s", [S, E], F32)

    with tile.TileContext(nc) as tc:
        with tc.tile_pool(name="consts", bufs=1) as consts:
            sb_bqk = consts.tile([P, JTQK], F32)
            nc.sync.dma_start(sb_bqk, bqk.rearrange("(jt p) -> p jt", p=P))
            sb_b1 = consts.tile([P, FC], F32)
            nc.sync.dma_start(sb_b1, b1e.rearrange("(f p) -> p f", p=P))
            sb_bvr = consts.tile([1, E], BF16)
            nc.sync.dma_start(sb_bvr, bvr[:, :])
            sb_bor = consts.tile([1, E], BF16)
            nc.sync.dma_start(sb_bor, bor[:, :])
            sb_b2r = consts.tile([1, E], BF16)
            nc.sync.dma_start(sb_b2r, b2r[:, :])
            sb_mask = consts.tile([P, 2, W], BF16)
            nc.sync.dma_start(sb_mask, maskadd.rearrange("(qt p) k -> p qt k", p=P))
            sb_id = consts.tile([P, P], BF16)
            nc.sync.dma_start(sb_id, id128[:, :])
            eps_t = consts.tile([P, 1], F32)
            nc.vector.memset(eps_t, EPS)
            ones_t = consts.tile([1, P], BF16)
            nc.vector.memset(ones_t, 1.0)

            # ---------------- stage B: attention block ----------------
            with (
                tc.tile_pool(name="wB", bufs=1) as wB,
                tc.tile_pool(name="xpool", bufs=6) as xpool,
                tc.tile_pool(name="x2spill", bufs=3) as x2spill,
                tc.tile_pool(name="sbB2", bufs=2) as sbB2,
                tc.tile_pool(name="sbB3", bufs=3) as sbB3,
                tc.tile_pool(name="sbB4", bufs=4) as sbB4,
                tc.tile_pool(name="sbB18", bufs=18) as sbB18,
                tc.tile_pool(name="ps256", bufs=3, space="PSUM") as ps256,
                tc.tile_pool(name="psmix", bufs=5, space="PSUM") as psmix,
            ):
                # qk weights in 4 column chunks (separate tiles) so the first
                # QK matmuls start after ~1MB instead of the full 8MB stream
                wqk_r = wqk.rearrange("(ec p) j -> p ec j", p=P)
                sb_wqk_c = []
                for c in range(4):
                    sb_wqk = wB.tile([P, EC, 512], BF16, name=f"sb_wqk{c}")
                    nc.sync.dma_start(sb_wqk, wqk_r[:, :, c * 512:(c + 1) * 512])
                    sb_wqk_c.append(sb_wqk)
                sb_wv = wB.tile([P, EC, E], BF16)
                nc.sync.dma_start(sb_wv, wv.rearrange("(ec p) j -> p ec j", p=P))
                sb_wo = wB.tile([P, EC, E], BF16)
                nc.sync.dma_start(sb_wo, wo.rearrange("(ec p) j -> p ec j", p=P))

                def ln1_transpose(win):
                    """Load x, LN1, transpose -> (x_tiles, x_T). Emitted mid
                    previous window so the chain overlaps attention."""
                    t0 = win * W
                    x_tiles = []
                    x_T = sbB2.tile([P, EC, W], BF16, tag="xT", name=f"x_T{win}")
                    for tt in range(2):
                        xt = xpool.tile([P, E], F32, tag="x", name=f"x{win}_{tt}")
                        nc.scalar.dma_start(xt, x_in[t0 + tt * P:t0 + (tt + 1) * P, :])
                        x_tiles.append(xt)
                        stats = sbB3.tile([P, 2, 6], F32, tag="bst")
                        for sg in range(2):
                            nc.vector.bn_stats(stats[:, sg], xt[:, sg * 512:(sg + 1) * 512])
                        mv = sbB3.tile([P, 2], F32, tag="mv")
                        nc.vector.bn_aggr(mv, stats)
                        std = sbB3.tile([P, 1], F32, tag="std")
                        nc.scalar.activation(std, mv[:, 1:2], AF.Sqrt, bias=eps_t)
                        rstd = sbB3.tile([P, 1], F32, tag="rstd")
                        nc.vector.reciprocal(rstd, std)
                        xnb = sbB4.tile([P, E], BF16, tag="xnb")
                        nc.vector.tensor_scalar(
                            out=xnb, in0=xt, scalar1=mv[:, 0:1], scalar2=rstd,
                            op0=ALU.subtract, op1=ALU.mult,
                        )
                        for ec2 in range(0, EC, 2):
                            ptr = psmix.tile([P, W], BF16, tag="mix", name="ptr_x")
                            nc.tensor.transpose(ptr[:, :P], xnb[:, ec2 * P:(ec2 + 1) * P], sb_id)
                            nc.tensor.transpose(ptr[:, P:], xnb[:, (ec2 + 1) * P:(ec2 + 2) * P], sb_id)
                            nc.vector.tensor_copy(
                                out=x_T[:, ec2:ec2 + 2, tt * P:(tt + 1) * P],
                                in_=ptr.rearrange("p (e q) -> p e q", e=2),
                            )
                    return x_tiles, x_T

                def qk_proj(x_T, win):
                    qkT = sbB2.tile([P, JTQK, W], BF16, tag="qkT", name=f"qkT{win}")
                    # q tile then matching k tile so scores of head pair h2 can
                    # start after 2 copies instead of 9
                    for idx in range(H // 2):
                        for jt in (idx, (H // 2) + idx):
                            ps = ps256.tile([P, W], F32, tag="mm256", name="ps_qk")
                            wq_c = sb_wqk_c[jt // 4]
                            jo = (jt % 4) * P
                            for ec in range(EC):
                                nc.tensor.matmul(
                                    ps, wq_c[:, ec, jo:jo + P], x_T[:, ec, :],
                                    start=(ec == 0), stop=(ec == EC - 1),
                                )
                            nc.scalar.activation(
                                qkT[:, jt, :], ps, AF.Prelu, bias=sb_bqk[:, jt:jt + 1], alpha=1.0,
                            )
                    return qkT

                def v_proj(x_T, win):
                    v_sb = sbB2.tile([P, 2, E], BF16, tag="v", name=f"v{win}")
                    for tt in range(2):
                        for jb in range(2):
                            ps = psmix.tile([P, 512], F32, tag="mix", name="ps_big")
                            for ec in range(EC):
                                nc.tensor.matmul(
                                    ps, x_T[:, ec, tt * P:(tt + 1) * P],
                                    sb_wv[:, ec, jb * 512:(jb + 1) * 512],
                                    start=(ec == 0), stop=False,
                                )
                            nc.tensor.matmul(
                                ps, ones_t, sb_bvr[0:1, jb * 512:(jb + 1) * 512],
                                start=False, stop=True,
                            )
                            nc.scalar.activation(v_sb[:, tt, jb * 512:(jb + 1) * 512], ps, AF.Copy)
                    return v_sb

                def attn_phase1(qkT):
                    # causal structure: qt0 rows attend only k<128 (upper k
                    # half fully masked -> zeros); qt1 rows attend all 256 with
                    # a triangular mask on the k 128..255 diagonal block only.
                    e_ns = []
                    for h in range(H):
                        h2, sub = h // 2, h % 2
                        poff = sub * DH
                        e_n = sbB18.tile([P, 2, W], BF16, tag="en", name=f"e_n{h}")
                        lsum = sbB18.tile([P, 2], F32, tag="l", name=f"l{h}")
                        ps0 = ps256.tile([P, W], F32, tag="mm256", name="ps_s0")
                        nc.tensor.matmul(ps0[:, :P], sb_id, sb_mask[:, 0, :P], start=True, stop=False)
                        nc.tensor.matmul(
                            ps0[:, :P],
                            qkT[poff:poff + DH, h2, :P],
                            qkT[poff:poff + DH, (H // 2) + h2, :P],
                            start=False, stop=True,
                        )
                        nc.scalar.activation(
                            e_n[:, 0, :P], ps0[:, :P], AF.Exp, scale=SCALE,
                            accum_out=lsum[:, 0:1],
                        )
                        nc.gpsimd.memset(e_n[:, 0, P:], 0.0)
                        ps1 = ps256.tile([P, W], F32, tag="mm256", name="ps_s1")
                        nc.tensor.matmul(
                            ps1,
                            qkT[poff:poff + DH, h2, P:],
                            qkT[poff:poff + DH, (H // 2) + h2, :],
                            start=True, stop=False,
                        )
                        nc.tensor.matmul(ps1[:, P:], sb_id, sb_mask[:, 0, :P], start=False, stop=True)
                        nc.scalar.activation(
                            e_n[:, 1, :], ps1, AF.Exp, scale=SCALE,
                            accum_out=lsum[:, 1:2],
                        )
                        rl = sbB18.tile([P, 2], F32, tag="rl", name=f"rl{h}")
                        nc.vector.reciprocal(rl, lsum)
                        nc.vector.tensor_scalar_mul(e_n[:, 0, :P], e_n[:, 0, :P], rl[:, 0:1])
                        nc.vector.tensor_scalar_mul(e_n[:, 1, :], e_n[:, 1, :], rl[:, 1:2])
                        e_ns.append(e_n)
                    return e_ns

                def attn_phase2(e_ns, v_sb):
                    eTs = []
                    for h in range(H):
                        e_n = e_ns[h]
                        eT = sbB18.tile([P, 2, W], BF16, tag="eT", name=f"eT{h}")
                        nc.gpsimd.memset(eT[:, 1, :P], 0.0)
                        ptr = psmix.tile([P, W], BF16, tag="mix", name="ptr_e")
                        nc.tensor.transpose(ptr[:, :P], e_n[:, 0, :P], sb_id)
                        nc.tensor.transpose(ptr[:, P:], e_n[:, 1, :P], sb_id)
                        nc.vector.tensor_copy(out=eT[:, 0, :], in_=ptr)
                        ptr2 = psmix.tile([P, W], BF16, tag="mix", name="ptr_e2")
                        nc.tensor.transpose(ptr2[:, :P], e_n[:, 1, P:], sb_id)
                        nc.vector.tensor_copy(out=eT[:, 1, P:], in_=ptr2[:, :P])
                        eTs.append(eT)
                    oT = sbB2.tile([P, EC, W], BF16, tag="oT")
                    for h2 in range(H // 2):
                        po = psmix.tile([P, W], F32, tag="mix", name="po")
                        for sub in range(2):
                            h = h2 * 2 + sub
                            poff = sub * DH
                            for kc in range(2):
                                nc.tensor.matmul(
                                    po[poff:poff + DH, :],
                                    v_sb[:, kc, h * DH:(h + 1) * DH],
                                    eTs[h][:, kc, :],
                                    start=(kc == 0), stop=(kc == 1),
                                    tile_position=(0, poff),
                                )
                        nc.vector.tensor_copy(out=oT[:, h2, :], in_=po)
                    return oT

                def out_proj(oT, x_tiles, win):
                    t0 = win * W
                    for tt in range(2):
                        x2t = x2spill.tile([P, E], F32, tag="x2")
                        for jb in range(2):
                            ps = psmix.tile([P, 512], F32, tag="mix", name="ps_big")
                            for ec in range(EC):
                                nc.tensor.matmul(
                                    ps, oT[:, ec, tt * P:(tt + 1) * P],
                                    sb_wo[:, ec, jb * 512:(jb + 1) * 512],
                                    start=(ec == 0), stop=False,
                                )
                            nc.tensor.matmul(
                                ps, ones_t, sb_bor[0:1, jb * 512:(jb + 1) * 512],
                                start=False, stop=True,
                            )
                            nc.vector.tensor_tensor(
                                out=x2t[:, jb * 512:(jb + 1) * 512], in0=ps,
                                in1=x_tiles[tt][:, jb * 512:(jb + 1) * 512], op=ALU.add,
                            )
                        nc.scalar.dma_start(x2s[t0 + tt * P:t0 + (tt + 1) * P, :], x2t)

                # steady state: phase1(w) first (highest priority), then
                # QK/V(w+1) as dense PE filler that the scheduler slots into
                # the softmax-latency stalls, then phase2(w) by which time all
                # of window w's attn weights are ready.
                ln_cur = ln1_transpose(0)
                qkT_cur = qk_proj(ln_cur[1], 0)
                v_cur = v_proj(ln_cur[1], 0)
                ln_nxt = ln1_transpose(1)
                for win in range(NWIN):
                    e_ns = attn_phase1(qkT_cur)
                    if win + 1 < NWIN:
                        qkT_nxt = qk_proj(ln_nxt[1], win + 1)
                        v_nxt = v_proj(ln_nxt[1], win + 1)
                        ln_n2 = ln1_transpose(win + 2) if win + 2 < NWIN else None
                    oT = attn_phase2(e_ns, v_cur)
                    out_proj(oT, ln_cur[0], win)
                    if win + 1 < NWIN:
                        ln_cur, qkT_cur, v_cur, ln_nxt = ln_nxt, qkT_nxt, v_nxt, ln_n2

            # ---------------- stage C: MLP block ----------------
            with (
                tc.tile_pool(name="wC", bufs=1) as wC,
                tc.tile_pool(name="x2pool", bufs=4) as x2pool,
                tc.tile_pool(name="sbC2", bufs=2) as sbC2,
                tc.tile_pool(name="sbC3", bufs=3) as sbC3,
                tc.tile_pool(name="sbC4", bufs=4) as sbC4,
                tc.tile_pool(name="psy", bufs=4, space="PSUM") as psy,
                tc.tile_pool(name="psu", bufs=2, space="PSUM") as psu_p,
                tc.tile_pool(name="pstr2", bufs=2, space="PSUM") as pstr2,
            ):
                sb_w1 = wC.tile([P, EC, 4 * E], BF16)
                nc.sync.dma_start(sb_w1, w1.rearrange("(ec p) f -> p ec f", p=P))
                sb_w2 = wC.tile([P, FC, E], BF16)
                nc.sync.dma_start(sb_w2, w2.rearrange("(f p) j -> p f j", p=P))

                def ln2_transpose(pair):
                    """Load x2, LN2, transpose -> (x2_tiles, h_T). Emitted one
                    pair ahead so the LN chain overlaps the previous f-loop."""
                    t0 = pair * W
                    x2_tiles = []
                    h_T = sbC2.tile([P, EC, W], BF16, tag="hT", name=f"h_T{pair}")
                    for tt in range(2):
                        x2t = x2pool.tile([P, E], F32, tag="x2c", name=f"x2c{pair}_{tt}")
                        nc.scalar.dma_start(x2t, x2s[t0 + tt * P:t0 + (tt + 1) * P, :])
                        x2_tiles.append(x2t)
                        stats = sbC3.tile([P, 2, 6], F32, tag="bst")
                        for sg in range(2):
                            nc.vector.bn_stats(stats[:, sg], x2t[:, sg * 512:(sg + 1) * 512])
                        mv = sbC3.tile([P, 2], F32, tag="mv")
                        nc.vector.bn_aggr(mv, stats)
                        std = sbC3.tile([P, 1], F32, tag="std")
                        nc.scalar.activation(std, mv[:, 1:2], AF.Sqrt, bias=eps_t)
                        rstd = sbC3.tile([P, 1], F32, tag="rstd")
                        nc.vector.reciprocal(rstd, std)
                        hnb = sbC4.tile([P, E], BF16, tag="hnb")
                        nc.vector.tensor_scalar(
                            out=hnb, in0=x2t, scalar1=mv[:, 0:1], scalar2=rstd,
                            op0=ALU.subtract, op1=ALU.mult,
                        )
                        for ec2 in range(0, EC, 2):
                            ptr = pstr2.tile([P, 2 * P], BF16, tag="tr2")
                            nc.tensor.transpose(ptr[:, :P], hnb[:, ec2 * P:(ec2 + 1) * P], sb_id)
                            nc.tensor.transpose(ptr[:, P:], hnb[:, (ec2 + 1) * P:(ec2 + 2) * P], sb_id)
                            nc.vector.tensor_copy(
                                out=h_T[:, ec2:ec2 + 2, tt * P:(tt + 1) * P],
                                in_=ptr.rearrange("p (e q) -> p e q", e=2),
                            )
                    return x2_tiles, h_T

                def mlp1(f, h_T):
                    psu = psu_p.tile([P, W], F32, tag="u", name=f"psu{f}")
                    for ec in range(EC):
                        nc.tensor.matmul(
                            psu, sb_w1[:, ec, f * P:(f + 1) * P], h_T[:, ec, :],
                            start=(ec == 0), stop=(ec == EC - 1),
                        )
                    g = sbC3.tile([P, W], BF16, tag="g", name=f"g{f}")
                    nc.scalar.activation(g, psu, AF.Gelu_apprx_tanh, bias=sb_b1[:, f:f + 1])
                    return g

                pending = ln2_transpose(0)
                for pair in range(NWIN):
                    t0 = pair * W
                    x2_tiles, h_T = pending

                    y_ps = [psy.tile([P, 512], F32, tag="y", name=f"y_ps{i}") for i in range(4)]
                    # software-pipelined f loop: MLP1(f+1) issued before MLP2(f);
                    # next pair's LN2+transpose chain emitted mid-loop so its
                    # DVE work hides under this pair's matmuls
                    g = mlp1(0, h_T)
                    for f in range(FC):
                        if f == 4 and pair + 1 < NWIN:
                            pending = ln2_transpose(pair + 1)
                        g_next = mlp1(f + 1, h_T) if f + 1 < FC else None
                        for tt in range(2):
                            for jb in range(2):
                                nc.tensor.matmul(
                                    y_ps[tt * 2 + jb], g[:, tt * P:(tt + 1) * P],
                                    sb_w2[:, f, jb * 512:(jb + 1) * 512],
                                    start=(f == 0), stop=False,
                                )
                        g = g_next
                    for tt in range(2):
                        yt = sbC3.tile([P, E], F32, tag="yt")
                        for jb in range(2):
                            nc.tensor.matmul(
                                y_ps[tt * 2 + jb], ones_t, sb_b2r[0:1, jb * 512:(jb + 1) * 512],
                                start=False, stop=True,
                            )
                            nc.vector.tensor_tensor(
                                out=yt[:, jb * 512:(jb + 1) * 512], in0=y_ps[tt * 2 + jb],
                                in1=x2_tiles[tt][:, jb * 512:(jb + 1) * 512], op=ALU.add,
                            )
                        nc.scalar.dma_start(y_out[t0 + tt * P:t0 + (tt + 1) * P, :], yt)

    nc.finalize()
    return nc


def get_nc():
    if "nc" not in _NC_CACHE:
        _NC_CACHE["nc"] = _build_nc()
    return _NC_CACHE["nc"]


def _prep_inputs(x, ln1_g, ln1_b, ln2_g, ln2_b, w_in, b_in, w_out, b_out, w1, b1, w2, b2):
    bf = ml_dtypes.bfloat16
    f32 = np.float32

    x = np.asarray(x, f32)
    ln1_g = np.asarray(ln1_g, f32)
    ln1_b = np.asarray(ln1_b, f32)
    ln2_g = np.asarray(ln2_g, f32)
    ln2_b = np.asarray(ln2_b, f32)
    w_in = np.asarray(w_in, f32)
    b_in = np.asarray(b_in, f32)
    w_out = np.asarray(w_out, f32)
    b_out = np.asarray(b_out, f32)
    w1 = np.asarray(w1, f32)
    b1 = np.asarray(b1, f32)
    w2 = np.asarray(w2, f32)
    b2 = np.asarray(b2, f32)

    # fold LN1 affine into the in-projection, LN2 affine into the MLP first layer
    wqk_r = w_in[:2 * E] * ln1_g[None, :]          # [2E, E]
    wv_r = w_in[2 * E:] * ln1_g[None, :]           # [E, E]
    bqk_eff = b_in[:2 * E] + w_in[:2 * E] @ ln1_b  # [2E]
    bv_eff = b_in[2 * E:] + w_in[2 * E:] @ ln1_b   # [E]
    w1_r = w1 * ln2_g[None, :]                     # [4E, E]
    b1_eff = b1 + w1 @ ln2_b                       # [4E]

    tril = np.tril(np.ones((W, W), bool))
    maskadd = np.where(tril, 0.0, NEG).astype(f32)

    shared = {
        "wqk": np.ascontiguousarray(wqk_r.T).astype(bf),
        "wv": np.ascontiguousarray(wv_r.T).astype(bf),
        "wo": np.ascontiguousarray(w_out.T).astype(bf),
        "w1": np.ascontiguousarray(w1_r.T).astype(bf),
        "w2": np.ascontiguousarray(w2.T).astype(bf),
        "bqk": np.ascontiguousarray(bqk_eff),
        "b1e": np.ascontiguousarray(b1_eff),
        "bvr": bv_eff.reshape(1, E).astype(bf),
        "bor": b_out.reshape(1, E).astype(bf),
        "b2r": b2.reshape(1, E).astype(bf),
        "maskadd": maskadd.astype(bf),
        "id128": np.eye(P, dtype=f32).astype(bf),
    }
    x_flat = np.ascontiguousarray(x.reshape(B * T, E))
    in_maps = []
    for i in range(NCORES):
        m = dict(shared)
        m["x"] = np.ascontiguousarray(x_flat[i * S:(i + 1) * S])
        in_maps.append(m)
    return in_maps


def kernel(**inputs) -> np.ndarray:
    in_maps = _prep_inputs(**inputs)
    nc = get_nc()
    res = run_bass_kernel_spmd(nc, in_maps, core_ids=list(range(NCORES)))
    out = np.concatenate([res.results[i]["y"] for i in range(NCORES)], axis=0)
    return np.ascontiguousarray(out.reshape(B, T, E))
